# revision 10
# baseline (speedup 1.0000x reference)
"""CAP-memory loss kernel for Trainium2 (8 NeuronCores).

The only heavy part of the reference is
    sims = normalize(features) @ normalize(mem0.reshape(C*L, D)).T     [B, C*L]
which streams the full 256 MB proxy memory. The C*L axis is sharded across
the 8 cores (camera c -> core c, 4096 rows each); each core runs a
DMA/PE-balanced fp8(e4m3) DoubleRow matmul over its 8 MB shard. The device
result is used ONLY to select top-k candidates, so each PSUM bank is
4:1 max-pooled on the DVE before it leaves the chip: the output stream is
[B, L/4] fp16 quad-maxima (512 KB/core instead of 2 MB), and the host
expands the top pooled quads back into element candidates.

Every value that enters the loss is recomputed exactly in f32 on the host:
  - per-camera CE logits: 8 x [32, 2048]x[2048, 4096] BLAS (~2 GFLOP),
    with the EMA-scatter columns patched from P = fn @ new_n.T (the
    scatter changes only B rows of the memory),
  - cross-camera positives and the BG_KNN hardest negatives: gathered and
    recomputed from a ~800-element quad-expanded shortlist (fp8 ranking
    noise << the shortlist margin), so the final loss matches the f32
    reference to ~1e-7 while the device stream is quarter-width.

Device schedule: every input DMA is enqueued up-front (no dma_start ever
carries a wait), the matmul stream runs gapless from the moment the first
tile lands, PSUM is split 4+4 across pass pairs so pooling of pass h
overlaps the matmuls of pass h+1, and the pooled output DMAs sit at the
end of the SP stream firing mid-kernel as their pools retire.
"""

import numpy as np

C, L, D = 8, 4096, 2048
B = 256
BETA = 0.05
ALPHA = 0.01
CROSSCAM_EPOCH = 5
BG_KNN = 50
N_CORES = 8
POOL_W = 4          # device max-pool window along the L axis
GAIN = 4.0          # fp8 range centering: device inputs are unit-norm * GAIN

_CACHE = {}


def _patch_tile_drain():
    """The walrus in this container rejects instructions with more than one
    sync wait; the stock TileContext exit puts every end-of-kernel wait on a
    single SP Drain. Spread them over dedicated single-wait nops instead."""
    import concourse.mybir as mybir
    import concourse.tile as tile
    from concourse.vector_clock import ScopedClock

    if getattr(tile.TileContext, "_drain_split_patch", False):
        return

    def _drain_and_barrier(self, tick_clock, wait_clock):
        nc = self.nc
        nop = nc.sync.nop(nofuse=True)
        wait_clock.add_sem_waits(
            nop.ins, ScopedClock({None: tick_clock.global_clock})
        )
        waits = list(nop.ins.sync_info.on_wait or [])
        if len(waits) > 1:
            nop.ins.sync_info = mybir.SyncInfo(on_wait=[waits[0]], on_update=[])
            for w in waits[1:]:
                extra = nc.sync.nop(nofuse=True)
                extra.ins.sync_info = mybir.SyncInfo(on_wait=[w], on_update=[])
        nc.sync.drain()
        nc.all_engine_barrier()
        assert self.sems is not None
        popped = nc._tile_sem_poison_stack.pop()
        assert popped is self._sem_poison
        nc.clear_and_free_semaphores(list(self.sems.allocated().values()))
        nc.all_engine_barrier()

    tile.TileContext._drain_and_barrier = _drain_and_barrier
    tile.TileContext._drain_split_patch = True


def _patch_tile_wait_split(max_waits=1):
    """This walrus rejects instructions carrying more than one sync wait.
    Before Tile lowers the scheduled instruction list, move extra waits onto
    same-engine NoOps inserted just before the offending instruction (engine
    queues are FIFO, so waiting earlier on the same engine is equivalent)."""
    import concourse.mybir as mybir
    import concourse.tile as tile

    if getattr(tile.TileContext, "_wait_split_patch", False):
        return
    orig = tile.TileContext._lower_ordered_insts
    counter = [0]

    def patched(self, ordered):
        for insts in ordered.values():
            new = []
            for inst in insts:
                try:
                    si = inst.sync_info
                    waits = list(si.on_wait or []) if si is not None else []
                except AttributeError:
                    waits = []
                if len(waits) > max_waits:
                    keep = waits[len(waits) - max_waits :]
                    for w in waits[: len(waits) - max_waits]:
                        counter[0] += 1
                        nop = mybir.InstNoOp(name=f"waitsplit-{counter[0]}")
                        nop.engine = inst.engine
                        nop.sync_info = mybir.SyncInfo(on_wait=[w], on_update=[])
                        new.append(nop)
                    inst.sync_info = mybir.SyncInfo(
                        on_wait=keep, on_update=list(si.on_update or [])
                    )
                new.append(inst)
            insts[:] = new
        return orig(self, ordered)

    tile.TileContext._lower_ordered_insts = patched
    tile.TileContext._wait_split_patch = True


def build_sims_program(
    Lsh=L, Dd=D, Bb=B, mm_dtype="float8e4", out_dtype="float16", double_row=True,
    pass_width=1024, pool_w=POOL_W,
):
    """Bass program: s0p[i, q] = max_{w<pool_w} sum_d fnT[d, i] * mT[d, q*pool_w+w].

    double_row=True (fp8 only): contraction chunks are 256 logical rows held
    as [128 partitions, 2] pairs (virtual 128x256 PE array, 2 MACs/cell/cyc).
    Logical row d = chunk*KROW + j*128 + p for both operands; any consistent
    (p, j) -> d mapping is valid since the cell computes w0*m0 + w1*m1.

    Inputs  fnT  [128, KC*PJ*Bb]  (normalized features * GAIN, host-chunked)
            mT   [Dd/PJ, PJ*Lsh]  (unit-norm memory shard * GAIN, host-chunked)
    Output  s0p  [Bb, Lsh/pool_w] (quad-max of scaled similarities, fp16)
    """
    import concourse.bass as bass
    import concourse.mybir as mybir
    import concourse.tile as tile

    _patch_tile_drain()
    _patch_tile_wait_split()
    dt = mybir.dt
    mmdt = getattr(dt, mm_dtype)
    outdt = getattr(dt, out_dtype)
    PJ = 2 if double_row else 1         # logical rows per partition element
    KROW = 128 * PJ
    perf_mode = mybir.MatmulPerfMode.DoubleRow if double_row else None

    assert Dd % KROW == 0 and Bb % 128 == 0 and Lsh % 512 == 0
    KC = Dd // KROW                     # contraction chunks
    NG = Bb // 128                      # output partition groups
    NH = Lsh // pass_width              # output column passes
    RS = pass_width // 512              # 512-wide psum banks per pass
    PWP = pass_width // pool_w          # pooled cols per pass
    BPW = 512 // pool_w                 # pooled cols per psum bank
    KF = 2                              # contraction chunks per fnT DMA
    assert NG * RS * 2 <= 8             # pass pairs double-buffer inside PSUM

    nc = bass.Bass()
    fnT_d = nc.declare_dram_parameter(
        "fnT", [128, KC * PJ * Bb], mmdt, isOutput=False
    )
    mT_d = nc.declare_dram_parameter("mT", [Dd // PJ, PJ * Lsh], mmdt, isOutput=False)
    s0_d = nc.declare_dram_parameter("s0", [Bb, Lsh // pool_w], outdt, isOutput=True)

    with tile.TileContext(nc) as tc:
        with (
            tc.tile_pool(name="const", bufs=1) as const_pool,
            tc.tile_pool(name="mt", bufs=10) as mt_pool,
            tc.tile_pool(name="tmp", bufs=2) as tmp_pool,
            tc.tile_pool(name="out", bufs=2) as out_pool,
            tc.tile_pool(name="psum", bufs=2, space="PSUM") as psum_pool,
        ):
            # HAM warm-up: the PE clock sits throttled at 1.2 GHz until the
            # activity monitor sees ~3.4us of sustained busy. Data can't
            # arrive before ~11us, so a burst of small zero matmuls starting
            # right at the entry barrier un-throttles the clock before the
            # real stream begins.
            warm = const_pool.tile([128, PJ, 128], mmdt, tag="warm")
            nc.gpsimd.memset(warm[:], 0.0)
            wps = psum_pool.tile([128, 512], dt.float32, tag="ps0_0", name="warm_ps")
            for _ in range(44):
                nc.tensor.matmul(
                    wps[:, :128], warm[:], warm[:],
                    start=True, stop=True, perf_mode=perf_mode,
                )

            # fnT arrives in KC/KF chunk tiles so the first matmul only waits
            # for chunk 0 (plus the first mt tile), not the full 512 KB
            fnT_view = fnT_d[:].rearrange("p (c j i) -> p c j i", c=KC, j=PJ)
            fnT_t = []
            for f in range(KC // KF):
                t = const_pool.tile([128, KF, PJ, Bb], mmdt, tag=f"fnT{f}")
                fnT_t.append(t)

            def dma_fnT(f, eng):
                eng.dma_start(fnT_t[f][:], fnT_view[:, f * KF : (f + 1) * KF])

            # enqueue the whole 8 MB input stream up-front, with the issue
            # cost (~600ns per dma_start) spread over the three DGE-capable
            # queues; the first tiles are in flight while the PE is still
            # warming up
            mt = {}

            def dma_mt(h, k, eng):
                t = mt_pool.tile(
                    [128, PJ, pass_width], mmdt, tag="mt", name=f"mt_{h}_{k}"
                )
                # host layout groups [h][j][r] per row, so this DMA reads
                # one contiguous PJ*pass_width run per partition
                eng.dma_start(
                    t[:],
                    mT_d[k * 128 : (k + 1) * 128, :].rearrange(
                        "p (h j r) -> p h j r", h=NH, j=PJ
                    )[:, h],
                )
                mt[h, k] = t

            dma_fnT(0, nc.sync)
            dma_mt(0, 0, nc.sync)
            dma_fnT(1, nc.sync)
            dma_mt(0, 1, nc.sync)
            for f in range(2, KC // KF):
                dma_fnT(f, nc.sync)
            rr = [nc.sync, nc.gpsimd, nc.scalar]
            i = 0
            for h in range(NH):
                for k in range(KC):
                    if (h, k) in ((0, 0), (0, 1)):
                        continue
                    dma_mt(h, k, rr[i % 3])
                    i += 1

            pending_out = []
            for h in range(NH):
                ps = {}
                for g in range(NG):
                    for rs in range(RS):
                        ps[g, rs] = psum_pool.tile(
                            [128, 512], dt.float32, tag=f"ps{g}_{rs}",
                            name=f"ps{g}_{rs}_{h}",
                        )
                outs = [
                    out_pool.tile(
                        [128, PWP], outdt, tag=f"out{g}", name=f"out{g}_{h}"
                    )
                    for g in range(NG)
                ]
                for k in range(KC):
                    for g in range(NG):
                        for rs in range(RS):
                            if double_row:
                                lhsT = fnT_t[k // KF][
                                    :, k % KF, :, g * 128 : (g + 1) * 128
                                ]
                                rhs = mt[h, k][:, :, rs * 512 : (rs + 1) * 512]
                            else:
                                lhsT = fnT_t[k // KF][
                                    :, k % KF, 0, g * 128 : (g + 1) * 128
                                ]
                                rhs = mt[h, k][:, 0, rs * 512 : (rs + 1) * 512]
                            nc.tensor.matmul(
                                ps[g, rs][:],
                                lhsT,
                                rhs,
                                start=(k == 0),
                                stop=(k == KC - 1),
                                perf_mode=perf_mode,
                            )
                for g in range(NG):
                    # 4:1 max-pool as the evacuation. The ISA allows at most
                    # one PSUM operand per DVE op, so the host interleaves
                    # each quad across the two banks of the pass: quad p sits
                    # at bank0[p], bank1[p], bank0[p+256], bank1[p+256].
                    t = tmp_pool.tile([128, 512], dt.float32, tag="t",
                                      name=f"t{g}_{h}")
                    u = tmp_pool.tile([128, 512], dt.float32, tag="u",
                                      name=f"u{g}_{h}")
                    nc.scalar.copy(t[:], ps[g, 0][:])
                    nc.vector.tensor_max(u[:], ps[g, 1][:], t[:])
                    nc.vector.tensor_max(
                        outs[g][:], u[:, 0:PWP], u[:, PWP : 2 * PWP]
                    )

                    # the last pass's two fires are the tail: split them over
                    # the two HW-DGE queues so they issue in parallel
                    feng = nc.scalar if (h == NH - 1 and g == 0) else nc.sync

                    def fire(g=g, h=h, outs=outs, feng=feng):
                        feng.dma_start(
                            s0_d[g * 128 : (g + 1) * 128, h * PWP : (h + 1) * PWP],
                            outs[g][:],
                        )
                    pending_out.append(fire)
            # all output DMAs at the end of the SP stream: each waits only on
            # its own pools, so they execute mid-kernel without ever blocking
            # an input tile behind them
            for fire in pending_out:
                fire()
    return nc


def _ensure_ntff_hook():
    """bass_utils' trace path imports antenv.axon_hooks, which this image's
    antenv lacks. Provide the module and register the ctypes NTFF hook the
    boot would have installed."""
    import sys
    import types

    try:
        import antenv.axon_hooks  # noqa: F401

        return
    except ImportError:
        pass
    import antenv

    mod = types.ModuleType("antenv.axon_hooks")
    state = {"h": None}
    mod.set_axon_ntff_profile_hook = lambda h: state.__setitem__("h", h)
    mod.get_axon_ntff_profile_hook = lambda: state["h"]
    sys.modules["antenv.axon_hooks"] = mod
    antenv.axon_hooks = mod
    try:
        from trn_agent_boot.trn_boot import _ntff_profile_via_ctypes

        h = _ntff_profile_via_ctypes("/opt/axon/libaxon_pjrt.so")
        if h is not None:
            mod.set_axon_ntff_profile_hook(h)
    except Exception:
        pass


def _get_program():
    if "nc" not in _CACHE:
        _CACHE["nc"] = build_sims_program()
    return _CACHE["nc"]


def _mm_np_dtype():
    import ml_dtypes

    return ml_dtypes.float8_e4m3


def _prep_mT(m, mmnp, n_pass=4):
    """[L, D] memory shard -> [D/2, 2*L] device layout: row (k*128+p) holds
    [h][j][r] so each (h, k) tile DMA is one contiguous run per partition;
    logical row d = k*256 + j*128 + p."""
    Lc, Dd = m.shape
    pw = Lc // n_pass
    return np.ascontiguousarray(
        m.T.reshape(Dd // 256, 2, 128, n_pass, pw)
        .transpose(0, 2, 3, 1, 4)
        .reshape(Dd // 2, 2 * Lc),
        dtype=mmnp,
    )


def _quad_perm():
    """Device row permutation: quad p of a pass must land at bank0[p],
    bank1[p], bank0[p+256], bank1[p+256] so the two pooling maxes only ever
    read one PSUM operand. perm[dev_row] = original row."""
    if "perm" in _CACHE:
        return _CACHE["perm"]
    j = np.arange(512)
    off = 4 * (j % 256) + 2 * (j // 256)
    passperm = np.concatenate([off, off + 1])         # dev col rs*512+j
    perm = (np.arange(L // 1024)[:, None] * 1024 + passperm[None, :]).reshape(-1)
    _CACHE["perm"] = perm
    return perm


def _device_sims_pooled(fn, mem0, invn_full):
    """fn [B, D] normalized; mem0 [C, L, D]; invn_full [C*L] reciprocal row
    norms. Returns pooled [B, C*L/POOL_W] f32: quad-max of GAIN^2-scaled
    normalized similarities, matmul + pooling on the 8 NeuronCores."""
    from concourse.bass_utils import run_bass_kernel_spmd

    nc = _get_program()
    Bb, Dd = fn.shape
    mmnp = _mm_np_dtype()
    # [D, B] -> [KC, 2, 128, B] -> [128, KC, 2, B] -> [128, KC*2*B]
    # (logical row d = c*256 + j*128 + p, pre-chunked for one linear DMA)
    fnT = np.ascontiguousarray(
        (fn.T * GAIN)
        .reshape(Dd // 256, 2, 128, Bb)
        .transpose(2, 0, 1, 3)
        .reshape(128, -1),
        dtype=mmnp,
    )
    perm = _quad_perm()
    in_maps = []
    for c in range(N_CORES):
        mn = mem0[c] * (invn_full[c * L : (c + 1) * L, None] * GAIN)
        in_maps.append({"fnT": fnT, "mT": _prep_mT(mn[perm], mmnp)})
    import os

    kwargs = {}
    if os.environ.get("KERNEL_TRACE"):
        _ensure_ntff_hook()
        kwargs = {"trace": True, "trace_cores": [0]}
    res = run_bass_kernel_spmd(nc, in_maps, core_ids=list(range(N_CORES)), **kwargs)
    _CACHE["exec_time_ns"] = res.exec_time_ns
    _CACHE["trace"] = res.instructions_and_trace
    return np.concatenate(
        [res.results[c]["s0"].astype(np.float32) for c in range(N_CORES)], axis=1
    )


def _logsumexp(x, axis):
    m = np.max(x, axis=axis, keepdims=True)
    return m + np.log(np.sum(np.exp(x - m), axis=axis, keepdims=True))


def _pool_maps():
    """Index maps between original L-columns and pooled quad indices.

    Device pooled column q of core c covers pass h = (q % 1024) // 256,
    quad p = q % 256 -> original columns c*4096 + h*1024 + 4p + {0..3}.
    """
    if "quad_l0" in _CACHE:
        return _CACHE["quad_l0"], _CACHE["col_to_quad"]
    nq = C * L // POOL_W
    q = np.arange(nq, dtype=np.int64)
    c, u = q // (L // POOL_W), q % (L // POOL_W)
    h, p = u // 256, u % 256
    quad_l0 = c * L + h * 1024 + POOL_W * p
    col_to_quad = np.empty(C * L, dtype=np.int64)
    cols = quad_l0[:, None] + np.arange(POOL_W)[None, :]
    col_to_quad[cols.reshape(-1)] = np.repeat(q, POOL_W)
    _CACHE["quad_l0"] = quad_l0
    _CACHE["col_to_quad"] = col_to_quad
    return quad_l0, col_to_quad


def kernel(
    features,
    targets,
    cams,
    all_pseudo_label,
    all_img_cams,
    init_intra_id_feat,
    epoch,
    batch_ind,
):
    f = np.asarray(features, dtype=np.float32)
    targets = np.asarray(targets)
    cams = np.asarray(cams)
    mem0 = np.asarray(init_intra_id_feat, dtype=np.float32)   # [C, L, D]
    percam = B // C

    fn = f / np.linalg.norm(f, axis=1, keepdims=True)
    mflat = mem0.reshape(C * L, D)
    invn_full = 1.0 / np.sqrt(np.einsum("rd,rd->r", mflat, mflat))

    # --- heavy part on device: quad-max-pooled normalized sims ---
    pooled = _device_sims_pooled(fn, mem0, invn_full)         # [B, C*L/4]

    # --- EMA update (only its effect on the CE logits is needed) ---
    old = mem0[cams, targets]                                 # [B, D]
    new = ALPHA * old + (1.0 - ALPHA) * f
    new_n = new / np.linalg.norm(new, axis=1, keepdims=True)
    # memn rows get normalized once more in the reference; idempotent but
    # replicate for exactness of the patched columns
    new_n = new_n / np.linalg.norm(new_n, axis=1, keepdims=True)
    P = fn @ new_n.T                                          # [B, B]

    # --- per-camera proxy CE; the diagonal blocks are recomputed exactly on
    # host (2 GFLOP in BLAS), independent of the device result ---
    logits = np.empty((C, percam, L), dtype=np.float32)
    for c in range(C):
        blk = (
            fn[c * percam : (c + 1) * percam] @ mflat[c * L : (c + 1) * L].T
        ) * invn_full[None, c * L : (c + 1) * L]
        for j in np.nonzero(cams == c)[0]:                    # scatter order: last wins
            blk[:, targets[j]] = P[c * percam : (c + 1) * percam, j]
        logits[c] = blk
    logits /= BETA
    lsm = logits - _logsumexp(logits, axis=-1)
    t = targets.reshape(C, percam)
    ce = -np.take_along_axis(lsm, t[..., None], axis=-1)[..., 0]
    loss = ce.mean(axis=1).sum()

    # --- cross-camera associative loss ---
    # The device ships only pooled quad-maxima; positives and the BG_KNN
    # hardest negatives are recomputed exactly on host from a quad-expanded
    # shortlist selected with a safety margin.
    if int(epoch) >= CROSSCAM_EPOCH:
        QP = 192
        quad_l0, col_to_quad = _pool_maps()
        pos = targets[:, None] + np.arange(C, dtype=np.int64)[None, :] * L
        m_pos = mflat[pos.reshape(-1)].reshape(B, C, D)
        pos_sims = (
            np.matmul(m_pos, fn[:, :, None])[..., 0] * invn_full[pos]
        )                                                     # [B, C] exact
        cand_q = np.argpartition(-pooled, QP - 1, axis=1)[:, :QP]     # [B, QP]
        quads = np.concatenate([cand_q, col_to_quad[pos]], axis=1)    # [B, QP+C]
        cand = (
            quad_l0[quads][:, :, None] + np.arange(POOL_W, dtype=np.int64)
        ).reshape(B, -1)                                      # [B, (QP+C)*4]
        cand = np.sort(cand, axis=1)
        m_c = mflat[cand.reshape(-1)].reshape(B, cand.shape[1], D)
        cvals = (
            np.matmul(m_c, fn[:, :, None])[..., 0] * invn_full[cand]
        )                                                     # exact
        # mask duplicate columns (a positive's quad may also be in the
        # top-QP) and the positive columns themselves
        dup = np.zeros_like(cand, dtype=bool)
        dup[:, 1:] = cand[:, 1:] == cand[:, :-1]
        is_pos = (cand[:, :, None] == pos[:, None, :]).any(axis=2)
        cvals[dup | is_pos] = -np.inf
        topv = -np.sort(-cvals, axis=1)[:, :BG_KNN]
        cat = np.concatenate([pos_sims / BETA, topv / BETA], axis=1).astype(
            np.float32
        )
        ls2 = cat - _logsumexp(cat, axis=1)
        per = -ls2[:, :C].sum(axis=1) / C
        loss = loss + 0.5 * per.reshape(C, percam).mean(axis=1).sum()

    return np.asarray([loss], dtype=np.float32)


# revision 12
# speedup vs baseline: 1.1157x; 1.1157x over previous
"""CAP-memory loss kernel for Trainium2 (8 NeuronCores).

The only heavy part of the reference is
    sims = normalize(features) @ normalize(mem0.reshape(C*L, D)).T     [B, C*L]
which streams the full 256 MB proxy memory. The C*L axis is sharded across
the 8 cores (camera c -> core c, 4096 rows each); each core runs a
DMA/PE-balanced fp8(e4m3) DoubleRow matmul over its 8 MB shard. The device
result is used ONLY to select top-k candidates, so each PSUM bank is
4:1 max-pooled on the DVE before it leaves the chip: the output stream is
[B, L/4] fp16 quad-maxima (512 KB/core instead of 2 MB), and the host
expands the top pooled quads back into element candidates.

Every value that enters the loss is recomputed exactly in f32 on the host:
  - per-camera CE logits: 8 x [32, 2048]x[2048, 4096] BLAS (~2 GFLOP),
    with the EMA-scatter columns patched from P = fn @ new_n.T (the
    scatter changes only B rows of the memory),
  - cross-camera positives and the BG_KNN hardest negatives: gathered and
    recomputed from a ~800-element quad-expanded shortlist (fp8 ranking
    noise << the shortlist margin), so the final loss matches the f32
    reference to ~1e-7 while the device stream is quarter-width.

Device schedule: every input DMA is enqueued up-front (no dma_start ever
carries a wait), the matmul stream runs gapless from the moment the first
tile lands, PSUM is split 4+4 across pass pairs so pooling of pass h
overlaps the matmuls of pass h+1, and the pooled output DMAs sit at the
end of the SP stream firing mid-kernel as their pools retire.
"""

import numpy as np

C, L, D = 8, 4096, 2048
B = 256
BETA = 0.05
ALPHA = 0.01
CROSSCAM_EPOCH = 5
BG_KNN = 50
N_CORES = 8
POOL_W = 4          # device max-pool window along the L axis
GAIN = 4.0          # fp8 range centering: device inputs are unit-norm * GAIN

_CACHE = {}


def _patch_tile_drain():
    """The walrus in this container rejects instructions with more than one
    sync wait; the stock TileContext exit puts every end-of-kernel wait on a
    single SP Drain. Spread them over dedicated single-wait nops instead."""
    import concourse.mybir as mybir
    import concourse.tile as tile
    from concourse.vector_clock import ScopedClock

    if getattr(tile.TileContext, "_drain_split_patch", False):
        return

    def _drain_and_barrier(self, tick_clock, wait_clock):
        nc = self.nc
        nop = nc.sync.nop(nofuse=True)
        wait_clock.add_sem_waits(
            nop.ins, ScopedClock({None: tick_clock.global_clock})
        )
        waits = list(nop.ins.sync_info.on_wait or [])
        if len(waits) > 1:
            nop.ins.sync_info = mybir.SyncInfo(on_wait=[waits[0]], on_update=[])
            for w in waits[1:]:
                extra = nc.sync.nop(nofuse=True)
                extra.ins.sync_info = mybir.SyncInfo(on_wait=[w], on_update=[])
        nc.sync.drain()
        nc.all_engine_barrier()
        assert self.sems is not None
        popped = nc._tile_sem_poison_stack.pop()
        assert popped is self._sem_poison
        nc.clear_and_free_semaphores(list(self.sems.allocated().values()))
        nc.all_engine_barrier()

    tile.TileContext._drain_and_barrier = _drain_and_barrier
    tile.TileContext._drain_split_patch = True


def _patch_tile_wait_split(max_waits=1):
    """This walrus rejects instructions carrying more than one sync wait.
    Before Tile lowers the scheduled instruction list, move extra waits onto
    same-engine NoOps inserted just before the offending instruction (engine
    queues are FIFO, so waiting earlier on the same engine is equivalent)."""
    import concourse.mybir as mybir
    import concourse.tile as tile

    if getattr(tile.TileContext, "_wait_split_patch", False):
        return
    orig = tile.TileContext._lower_ordered_insts
    counter = [0]

    def patched(self, ordered):
        for insts in ordered.values():
            new = []
            for inst in insts:
                try:
                    si = inst.sync_info
                    waits = list(si.on_wait or []) if si is not None else []
                except AttributeError:
                    waits = []
                if len(waits) > max_waits:
                    keep = waits[len(waits) - max_waits :]
                    for w in waits[: len(waits) - max_waits]:
                        counter[0] += 1
                        nop = mybir.InstNoOp(name=f"waitsplit-{counter[0]}")
                        nop.engine = inst.engine
                        nop.sync_info = mybir.SyncInfo(on_wait=[w], on_update=[])
                        new.append(nop)
                    inst.sync_info = mybir.SyncInfo(
                        on_wait=keep, on_update=list(si.on_update or [])
                    )
                new.append(inst)
            insts[:] = new
        return orig(self, ordered)

    tile.TileContext._lower_ordered_insts = patched
    tile.TileContext._wait_split_patch = True


def build_sims_program(
    Lsh=L, Dd=D, Bb=B, mm_dtype="float8e4", out_dtype="float16", double_row=True,
    pass_width=1024, pool_w=POOL_W,
):
    """Bass program: s0p[i, q] = max_{w<pool_w} sum_d fnT[d, i] * mT[d, q*pool_w+w].

    double_row=True (fp8 only): contraction chunks are 256 logical rows held
    as [128 partitions, 2] pairs (virtual 128x256 PE array, 2 MACs/cell/cyc).
    Logical row d = chunk*KROW + j*128 + p for both operands; any consistent
    (p, j) -> d mapping is valid since the cell computes w0*m0 + w1*m1.

    Inputs  fnT  [128, KC*PJ*Bb]  (normalized features * GAIN, host-chunked)
            mT   [Dd/PJ, PJ*Lsh]  (unit-norm memory shard * GAIN, host-chunked)
    Output  s0p  [Bb, Lsh/pool_w] (quad-max of scaled similarities, fp16)
    """
    import concourse.bass as bass
    import concourse.mybir as mybir
    import concourse.tile as tile

    _patch_tile_drain()
    _patch_tile_wait_split()
    dt = mybir.dt
    mmdt = getattr(dt, mm_dtype)
    outdt = getattr(dt, out_dtype)
    PJ = 2 if double_row else 1         # logical rows per partition element
    KROW = 128 * PJ
    perf_mode = mybir.MatmulPerfMode.DoubleRow if double_row else None

    assert Dd % KROW == 0 and Bb % 128 == 0 and Lsh % 512 == 0
    KC = Dd // KROW                     # contraction chunks
    NG = Bb // 128                      # output partition groups
    NH = Lsh // pass_width              # output column passes
    RS = pass_width // 512              # 512-wide psum banks per pass
    PWP = pass_width // pool_w          # pooled cols per pass
    BPW = 512 // pool_w                 # pooled cols per psum bank
    KF = 2                              # contraction chunks per fnT DMA
    assert NG * RS * 2 <= 8             # pass pairs double-buffer inside PSUM

    nc = bass.Bass()
    fnT_d = nc.declare_dram_parameter(
        "fnT", [128, KC * PJ * Bb], mmdt, isOutput=False
    )
    mT_d = nc.declare_dram_parameter("mT", [Dd // PJ, PJ * Lsh], mmdt, isOutput=False)
    s0_d = nc.declare_dram_parameter("s0", [Bb, Lsh // pool_w], outdt, isOutput=True)

    with tile.TileContext(nc) as tc:
        with (
            tc.tile_pool(name="const", bufs=1) as const_pool,
            tc.tile_pool(name="mt", bufs=NH * KC) as mt_pool,
            tc.tile_pool(name="tmp", bufs=2) as tmp_pool,
            tc.tile_pool(name="out", bufs=4) as out_pool,
            tc.tile_pool(name="psum", bufs=2, space="PSUM") as psum_pool,
        ):
            # HAM warm-up: the PE clock sits throttled at 1.2 GHz until the
            # activity monitor sees ~3.4us of sustained busy. Data can't
            # arrive before ~11us, so a burst of small zero matmuls starting
            # right at the entry barrier un-throttles the clock before the
            # real stream begins.
            warm = const_pool.tile([128, PJ, 128], mmdt, tag="warm")
            nc.gpsimd.memset(warm[:], 0.0)
            wps = psum_pool.tile([128, 512], dt.float32, tag="ps0_0", name="warm_ps")
            for _ in range(44):
                nc.tensor.matmul(
                    wps[:, :128], warm[:], warm[:],
                    start=True, stop=True, perf_mode=perf_mode,
                )

            # fnT arrives in KC/KF chunk tiles so the first matmul only waits
            # for chunk 0 (plus the first mt tile), not the full 512 KB
            fnT_view = fnT_d[:].rearrange("p (c j i) -> p c j i", c=KC, j=PJ)
            fnT_t = []
            for f in range(KC // KF):
                t = const_pool.tile([128, KF, PJ, Bb], mmdt, tag=f"fnT{f}")
                fnT_t.append(t)

            def dma_fnT(f, eng):
                eng.dma_start(fnT_t[f][:], fnT_view[:, f * KF : (f + 1) * KF])

            # enqueue the whole 8 MB input stream up-front, with the issue
            # cost (~600ns per dma_start) spread over the three DGE-capable
            # queues; the first tiles are in flight while the PE is still
            # warming up
            mt = {}

            def dma_mt(h, k, eng):
                t = mt_pool.tile(
                    [128, PJ, pass_width], mmdt, tag="mt", name=f"mt_{h}_{k}"
                )
                # host layout groups [h][j][r] per row, so this DMA reads
                # one contiguous PJ*pass_width run per partition
                eng.dma_start(
                    t[:],
                    mT_d[k * 128 : (k + 1) * 128, :].rearrange(
                        "p (h j r) -> p h j r", h=NH, j=PJ
                    )[:, h],
                )
                mt[h, k] = t

            # critical-path transfers (everything the first two matmul groups
            # need) go on SP alone so ACT traffic can't delay them in the
            # rings; the rest alternates SP/ACT. With one pool buffer per
            # tile no dma_start carries a wait, so both queues burst all
            # issues out in the first ~10us. GpSimd SW-DGE is avoided
            # entirely (its drain costs ~7us in the epilogue).
            dma_fnT(0, nc.sync)
            dma_mt(0, 0, nc.sync)
            dma_fnT(1, nc.sync)
            dma_mt(0, 1, nc.sync)
            for f in range(2, KC // KF):
                dma_fnT(f, nc.sync)
            rr = [nc.sync, nc.scalar]
            i = 0
            for h in range(NH):
                for k in range(KC):
                    if (h, k) in ((0, 0), (0, 1)):
                        continue
                    dma_mt(h, k, rr[i % 2])
                    i += 1

            pending_out = []
            for h in range(NH):
                ps = {}
                for g in range(NG):
                    for rs in range(RS):
                        ps[g, rs] = psum_pool.tile(
                            [128, 512], dt.float32, tag=f"ps{g}_{rs}",
                            name=f"ps{g}_{rs}_{h}",
                        )
                outs = [
                    out_pool.tile(
                        [128, PWP], outdt, tag=f"out{g}", name=f"out{g}_{h}"
                    )
                    for g in range(NG)
                ]
                for k in range(KC):
                    for g in range(NG):
                        for rs in range(RS):
                            if double_row:
                                lhsT = fnT_t[k // KF][
                                    :, k % KF, :, g * 128 : (g + 1) * 128
                                ]
                                rhs = mt[h, k][:, :, rs * 512 : (rs + 1) * 512]
                            else:
                                lhsT = fnT_t[k // KF][
                                    :, k % KF, 0, g * 128 : (g + 1) * 128
                                ]
                                rhs = mt[h, k][:, 0, rs * 512 : (rs + 1) * 512]
                            nc.tensor.matmul(
                                ps[g, rs][:],
                                lhsT,
                                rhs,
                                start=(k == 0),
                                stop=(k == KC - 1),
                                perf_mode=perf_mode,
                            )
                for g in range(NG):
                    # 4:1 max-pool as the evacuation. The ISA allows at most
                    # one PSUM operand per DVE op, so the host interleaves
                    # each quad across the two banks of the pass: quad p sits
                    # at bank0[p], bank1[p], bank0[p+256], bank1[p+256].
                    t = tmp_pool.tile([128, 512], dt.float32, tag="t",
                                      name=f"t{g}_{h}")
                    u = tmp_pool.tile([128, 512], dt.float32, tag="u",
                                      name=f"u{g}_{h}")
                    nc.scalar.copy(t[:], ps[g, 0][:])
                    nc.vector.tensor_max(u[:], ps[g, 1][:], t[:])
                    nc.vector.tensor_max(
                        outs[g][:], u[:, 0:PWP], u[:, PWP : 2 * PWP]
                    )

                    # the last pass's two fires are the tail: split them over
                    # the two HW-DGE queues so they issue in parallel
                    feng = nc.scalar if (h == NH - 1 and g == 0) else nc.sync

                    def fire(g=g, h=h, outs=outs, feng=feng):
                        feng.dma_start(
                            s0_d[g * 128 : (g + 1) * 128, h * PWP : (h + 1) * PWP],
                            outs[g][:],
                        )
                    pending_out.append(fire)
            # all output DMAs at the end of the SP stream: each waits only on
            # its own pools, so they execute mid-kernel without ever blocking
            # an input tile behind them
            for fire in pending_out:
                fire()
    return nc


def _ensure_ntff_hook():
    """bass_utils' trace path imports antenv.axon_hooks, which this image's
    antenv lacks. Provide the module and register the ctypes NTFF hook the
    boot would have installed."""
    import sys
    import types

    try:
        import antenv.axon_hooks  # noqa: F401

        return
    except ImportError:
        pass
    import antenv

    mod = types.ModuleType("antenv.axon_hooks")
    state = {"h": None}
    mod.set_axon_ntff_profile_hook = lambda h: state.__setitem__("h", h)
    mod.get_axon_ntff_profile_hook = lambda: state["h"]
    sys.modules["antenv.axon_hooks"] = mod
    antenv.axon_hooks = mod
    try:
        from trn_agent_boot.trn_boot import _ntff_profile_via_ctypes

        h = _ntff_profile_via_ctypes("/opt/axon/libaxon_pjrt.so")
        if h is not None:
            mod.set_axon_ntff_profile_hook(h)
    except Exception:
        pass


def _get_program():
    if "nc" not in _CACHE:
        _CACHE["nc"] = build_sims_program()
    return _CACHE["nc"]


def _mm_np_dtype():
    import ml_dtypes

    return ml_dtypes.float8_e4m3


def _prep_mT(m, mmnp, n_pass=4):
    """[L, D] memory shard -> [D/2, 2*L] device layout: row (k*128+p) holds
    [h][j][r] so each (h, k) tile DMA is one contiguous run per partition;
    logical row d = k*256 + j*128 + p."""
    Lc, Dd = m.shape
    pw = Lc // n_pass
    return np.ascontiguousarray(
        m.T.reshape(Dd // 256, 2, 128, n_pass, pw)
        .transpose(0, 2, 3, 1, 4)
        .reshape(Dd // 2, 2 * Lc),
        dtype=mmnp,
    )


def _quad_perm():
    """Device row permutation: quad p of a pass must land at bank0[p],
    bank1[p], bank0[p+256], bank1[p+256] so the two pooling maxes only ever
    read one PSUM operand. perm[dev_row] = original row."""
    if "perm" in _CACHE:
        return _CACHE["perm"]
    j = np.arange(512)
    off = 4 * (j % 256) + 2 * (j // 256)
    passperm = np.concatenate([off, off + 1])         # dev col rs*512+j
    perm = (np.arange(L // 1024)[:, None] * 1024 + passperm[None, :]).reshape(-1)
    _CACHE["perm"] = perm
    return perm


def _device_sims_pooled(fn, mem0, invn_full):
    """fn [B, D] normalized; mem0 [C, L, D]; invn_full [C*L] reciprocal row
    norms. Returns pooled [B, C*L/POOL_W] f32: quad-max of GAIN^2-scaled
    normalized similarities, matmul + pooling on the 8 NeuronCores."""
    from concourse.bass_utils import run_bass_kernel_spmd

    nc = _get_program()
    Bb, Dd = fn.shape
    mmnp = _mm_np_dtype()
    # [D, B] -> [KC, 2, 128, B] -> [128, KC, 2, B] -> [128, KC*2*B]
    # (logical row d = c*256 + j*128 + p, pre-chunked for one linear DMA)
    fnT = np.ascontiguousarray(
        (fn.T * GAIN)
        .reshape(Dd // 256, 2, 128, Bb)
        .transpose(2, 0, 1, 3)
        .reshape(128, -1),
        dtype=mmnp,
    )
    perm = _quad_perm()
    in_maps = []
    for c in range(N_CORES):
        mn = mem0[c] * (invn_full[c * L : (c + 1) * L, None] * GAIN)
        in_maps.append({"fnT": fnT, "mT": _prep_mT(mn[perm], mmnp)})
    import os

    kwargs = {}
    if os.environ.get("KERNEL_TRACE"):
        _ensure_ntff_hook()
        kwargs = {"trace": True, "trace_cores": [0]}
    res = run_bass_kernel_spmd(nc, in_maps, core_ids=list(range(N_CORES)), **kwargs)
    _CACHE["exec_time_ns"] = res.exec_time_ns
    _CACHE["trace"] = res.instructions_and_trace
    return np.concatenate(
        [res.results[c]["s0"].astype(np.float32) for c in range(N_CORES)], axis=1
    )


def _logsumexp(x, axis):
    m = np.max(x, axis=axis, keepdims=True)
    return m + np.log(np.sum(np.exp(x - m), axis=axis, keepdims=True))


def _pool_maps():
    """Index maps between original L-columns and pooled quad indices.

    Device pooled column q of core c covers pass h = (q % 1024) // 256,
    quad p = q % 256 -> original columns c*4096 + h*1024 + 4p + {0..3}.
    """
    if "quad_l0" in _CACHE:
        return _CACHE["quad_l0"], _CACHE["col_to_quad"]
    nq = C * L // POOL_W
    q = np.arange(nq, dtype=np.int64)
    c, u = q // (L // POOL_W), q % (L // POOL_W)
    h, p = u // 256, u % 256
    quad_l0 = c * L + h * 1024 + POOL_W * p
    col_to_quad = np.empty(C * L, dtype=np.int64)
    cols = quad_l0[:, None] + np.arange(POOL_W)[None, :]
    col_to_quad[cols.reshape(-1)] = np.repeat(q, POOL_W)
    _CACHE["quad_l0"] = quad_l0
    _CACHE["col_to_quad"] = col_to_quad
    return quad_l0, col_to_quad


def kernel(
    features,
    targets,
    cams,
    all_pseudo_label,
    all_img_cams,
    init_intra_id_feat,
    epoch,
    batch_ind,
):
    f = np.asarray(features, dtype=np.float32)
    targets = np.asarray(targets)
    cams = np.asarray(cams)
    mem0 = np.asarray(init_intra_id_feat, dtype=np.float32)   # [C, L, D]
    percam = B // C

    fn = f / np.linalg.norm(f, axis=1, keepdims=True)
    mflat = mem0.reshape(C * L, D)
    invn_full = 1.0 / np.sqrt(np.einsum("rd,rd->r", mflat, mflat))

    # --- heavy part on device: quad-max-pooled normalized sims ---
    pooled = _device_sims_pooled(fn, mem0, invn_full)         # [B, C*L/4]

    # --- EMA update (only its effect on the CE logits is needed) ---
    old = mem0[cams, targets]                                 # [B, D]
    new = ALPHA * old + (1.0 - ALPHA) * f
    new_n = new / np.linalg.norm(new, axis=1, keepdims=True)
    # memn rows get normalized once more in the reference; idempotent but
    # replicate for exactness of the patched columns
    new_n = new_n / np.linalg.norm(new_n, axis=1, keepdims=True)
    P = fn @ new_n.T                                          # [B, B]

    # --- per-camera proxy CE; the diagonal blocks are recomputed exactly on
    # host (2 GFLOP in BLAS), independent of the device result ---
    logits = np.empty((C, percam, L), dtype=np.float32)
    for c in range(C):
        blk = (
            fn[c * percam : (c + 1) * percam] @ mflat[c * L : (c + 1) * L].T
        ) * invn_full[None, c * L : (c + 1) * L]
        for j in np.nonzero(cams == c)[0]:                    # scatter order: last wins
            blk[:, targets[j]] = P[c * percam : (c + 1) * percam, j]
        logits[c] = blk
    logits /= BETA
    lsm = logits - _logsumexp(logits, axis=-1)
    t = targets.reshape(C, percam)
    ce = -np.take_along_axis(lsm, t[..., None], axis=-1)[..., 0]
    loss = ce.mean(axis=1).sum()

    # --- cross-camera associative loss ---
    # The device ships only pooled quad-maxima; positives and the BG_KNN
    # hardest negatives are recomputed exactly on host from a quad-expanded
    # shortlist selected with a safety margin.
    if int(epoch) >= CROSSCAM_EPOCH:
        QP = 192
        quad_l0, col_to_quad = _pool_maps()
        pos = targets[:, None] + np.arange(C, dtype=np.int64)[None, :] * L
        m_pos = mflat[pos.reshape(-1)].reshape(B, C, D)
        pos_sims = (
            np.matmul(m_pos, fn[:, :, None])[..., 0] * invn_full[pos]
        )                                                     # [B, C] exact
        cand_q = np.argpartition(-pooled, QP - 1, axis=1)[:, :QP]     # [B, QP]
        quads = np.concatenate([cand_q, col_to_quad[pos]], axis=1)    # [B, QP+C]
        cand = (
            quad_l0[quads][:, :, None] + np.arange(POOL_W, dtype=np.int64)
        ).reshape(B, -1)                                      # [B, (QP+C)*4]
        cand = np.sort(cand, axis=1)
        m_c = mflat[cand.reshape(-1)].reshape(B, cand.shape[1], D)
        cvals = (
            np.matmul(m_c, fn[:, :, None])[..., 0] * invn_full[cand]
        )                                                     # exact
        # mask duplicate columns (a positive's quad may also be in the
        # top-QP) and the positive columns themselves
        dup = np.zeros_like(cand, dtype=bool)
        dup[:, 1:] = cand[:, 1:] == cand[:, :-1]
        is_pos = (cand[:, :, None] == pos[:, None, :]).any(axis=2)
        cvals[dup | is_pos] = -np.inf
        topv = -np.sort(-cvals, axis=1)[:, :BG_KNN]
        cat = np.concatenate([pos_sims / BETA, topv / BETA], axis=1).astype(
            np.float32
        )
        ls2 = cat - _logsumexp(cat, axis=1)
        per = -ls2[:, :C].sum(axis=1) / C
        loss = loss + 0.5 * per.reshape(C, percam).mean(axis=1).sum()

    return np.asarray([loss], dtype=np.float32)


# revision 15
# speedup vs baseline: 1.1781x; 1.0560x over previous
"""CAP-memory loss kernel for Trainium2 (8 NeuronCores).

The only heavy part of the reference is
    sims = normalize(features) @ normalize(mem0.reshape(C*L, D)).T     [B, C*L]
which streams the full 256 MB proxy memory. The C*L axis is sharded across
the 8 cores (camera c -> core c, 4096 rows each); each core runs a
DMA/PE-balanced fp8(e4m3) DoubleRow matmul over its 8 MB shard. The device
result is used ONLY to select top-k candidates, so each PSUM bank is
4:1 max-pooled on the DVE before it leaves the chip: the output stream is
[B, L/4] fp16 quad-maxima (512 KB/core instead of 2 MB), and the host
expands the top pooled quads back into element candidates.

Every value that enters the loss is recomputed exactly in f32 on the host:
  - per-camera CE logits: 8 x [32, 2048]x[2048, 4096] BLAS (~2 GFLOP),
    with the EMA-scatter columns patched from P = fn @ new_n.T (the
    scatter changes only B rows of the memory),
  - cross-camera positives and the BG_KNN hardest negatives: gathered and
    recomputed from a ~800-element quad-expanded shortlist (fp8 ranking
    noise << the shortlist margin), so the final loss matches the f32
    reference to ~1e-7 while the device stream is quarter-width.

Device schedule: every input DMA is enqueued up-front (no dma_start ever
carries a wait), the matmul stream runs gapless from the moment the first
tile lands, PSUM is split 4+4 across pass pairs so pooling of pass h
overlaps the matmuls of pass h+1, and the pooled output DMAs sit at the
end of the SP stream firing mid-kernel as their pools retire.
"""

import numpy as np

C, L, D = 8, 4096, 2048
B = 256
BETA = 0.05
ALPHA = 0.01
CROSSCAM_EPOCH = 5
BG_KNN = 50
N_CORES = 8
POOL_W = 4          # device max-pool window along the L axis
GAIN = 4.0          # fp8 range centering: device inputs are unit-norm * GAIN

_CACHE = {}


def _patch_tile_drain():
    """The walrus in this container rejects instructions with more than one
    sync wait; the stock TileContext exit puts every end-of-kernel wait on a
    single SP Drain. Spread them over dedicated single-wait nops instead."""
    import concourse.mybir as mybir
    import concourse.tile as tile
    from concourse.vector_clock import ScopedClock

    if getattr(tile.TileContext, "_drain_split_patch", False):
        return

    def _drain_and_barrier(self, tick_clock, wait_clock):
        nc = self.nc
        nop = nc.sync.nop(nofuse=True)
        wait_clock.add_sem_waits(
            nop.ins, ScopedClock({None: tick_clock.global_clock})
        )
        waits = list(nop.ins.sync_info.on_wait or [])
        if len(waits) > 1:
            nop.ins.sync_info = mybir.SyncInfo(on_wait=[waits[0]], on_update=[])
            for w in waits[1:]:
                extra = nc.sync.nop(nofuse=True)
                extra.ins.sync_info = mybir.SyncInfo(on_wait=[w], on_update=[])
        nc.sync.drain()
        nc.all_engine_barrier()
        assert self.sems is not None
        popped = nc._tile_sem_poison_stack.pop()
        assert popped is self._sem_poison
        nc.clear_and_free_semaphores(list(self.sems.allocated().values()))
        nc.all_engine_barrier()

    tile.TileContext._drain_and_barrier = _drain_and_barrier
    tile.TileContext._drain_split_patch = True


def _patch_tile_wait_split(max_waits=1):
    """This walrus rejects instructions carrying more than one sync wait.
    Before Tile lowers the scheduled instruction list, move extra waits onto
    same-engine NoOps inserted just before the offending instruction (engine
    queues are FIFO, so waiting earlier on the same engine is equivalent)."""
    import concourse.mybir as mybir
    import concourse.tile as tile

    if getattr(tile.TileContext, "_wait_split_patch", False):
        return
    orig = tile.TileContext._lower_ordered_insts
    counter = [0]

    def patched(self, ordered):
        for insts in ordered.values():
            new = []
            for inst in insts:
                try:
                    si = inst.sync_info
                    waits = list(si.on_wait or []) if si is not None else []
                except AttributeError:
                    waits = []
                if len(waits) > max_waits:
                    keep = waits[len(waits) - max_waits :]
                    for w in waits[: len(waits) - max_waits]:
                        counter[0] += 1
                        nop = mybir.InstNoOp(name=f"waitsplit-{counter[0]}")
                        nop.engine = inst.engine
                        nop.sync_info = mybir.SyncInfo(on_wait=[w], on_update=[])
                        new.append(nop)
                    inst.sync_info = mybir.SyncInfo(
                        on_wait=keep, on_update=list(si.on_update or [])
                    )
                new.append(inst)
            insts[:] = new
        return orig(self, ordered)

    tile.TileContext._lower_ordered_insts = patched
    tile.TileContext._wait_split_patch = True


def build_sims_program(
    Lsh=L, Dd=D, Bb=B, mm_dtype="float8e4", out_dtype="float16", double_row=True,
    pass_width=1024, pool_w=POOL_W,
):
    """Bass program: s0p[i, q] = max_{w<pool_w} sum_d fnT[d, i] * mT[d, q*pool_w+w].

    double_row=True (fp8 only): contraction chunks are 256 logical rows held
    as [128 partitions, 2] pairs (virtual 128x256 PE array, 2 MACs/cell/cyc).
    Logical row d = chunk*KROW + j*128 + p for both operands; any consistent
    (p, j) -> d mapping is valid since the cell computes w0*m0 + w1*m1.

    Inputs  fnT  [128, KC*PJ*Bb]  (normalized features * GAIN, host-chunked)
            mT   [Dd/PJ, PJ*Lsh]  (unit-norm memory shard * GAIN, host-chunked)
    Output  s0p  [Bb, Lsh/pool_w] (quad-max of scaled similarities, fp16)
    """
    import concourse.bass as bass
    import concourse.mybir as mybir
    import concourse.tile as tile

    _patch_tile_drain()
    _patch_tile_wait_split()
    dt = mybir.dt
    mmdt = getattr(dt, mm_dtype)
    outdt = getattr(dt, out_dtype)
    PJ = 2 if double_row else 1         # logical rows per partition element
    KROW = 128 * PJ
    perf_mode = mybir.MatmulPerfMode.DoubleRow if double_row else None

    assert Dd % KROW == 0 and Bb % 128 == 0 and Lsh % 512 == 0
    KC = Dd // KROW                     # contraction chunks
    NG = Bb // 128                      # output partition groups
    NH = Lsh // pass_width              # output column passes
    RS = pass_width // 512              # 512-wide psum banks per pass
    PWP = pass_width // pool_w          # pooled cols per pass
    BPW = 512 // pool_w                 # pooled cols per psum bank
    KF = 2                              # contraction chunks per fnT DMA
    assert NG * RS * 2 <= 8             # pass pairs double-buffer inside PSUM

    nc = bass.Bass()
    fnT_d = nc.declare_dram_parameter(
        "fnT", [128, KC * PJ * Bb], mmdt, isOutput=False
    )
    mT_d = nc.declare_dram_parameter("mT", [Dd // PJ, PJ * Lsh], mmdt, isOutput=False)
    s0_d = nc.declare_dram_parameter("s0", [Bb, Lsh // pool_w], outdt, isOutput=True)

    with tile.TileContext(nc) as tc:
        with (
            tc.tile_pool(name="const", bufs=1) as const_pool,
            tc.tile_pool(name="mt", bufs=NH * KC) as mt_pool,
            tc.tile_pool(name="tmp", bufs=2) as tmp_pool,
            tc.tile_pool(name="out", bufs=4) as out_pool,
            tc.tile_pool(name="psum", bufs=2, space="PSUM") as psum_pool,
        ):
            # HAM warm-up: the PE clock sits throttled at 1.2 GHz until the
            # activity monitor sees ~3.4us of sustained busy. Data can't
            # arrive before ~11us, so a burst of small zero matmuls starting
            # right at the entry barrier un-throttles the clock before the
            # real stream begins.
            warm = const_pool.tile([128, PJ, 128], mmdt, tag="warm")
            nc.gpsimd.memset(warm[:], 0.0)
            wps = psum_pool.tile([128, 512], dt.float32, tag="ps0_0", name="warm_ps")
            for _ in range(44):
                nc.tensor.matmul(
                    wps[:, :128], warm[:], warm[:],
                    start=True, stop=True, perf_mode=perf_mode,
                )

            # fnT arrives in KC/KF chunk tiles so the first matmul only waits
            # for chunk 0 (plus the first mt tile), not the full 512 KB
            fnT_view = fnT_d[:].rearrange("p (c j i) -> p c j i", c=KC, j=PJ)
            fnT_t = []
            for f in range(KC // KF):
                t = const_pool.tile([128, KF, PJ, Bb], mmdt, tag=f"fnT{f}")
                fnT_t.append(t)

            def dma_fnT(f, eng):
                eng.dma_start(fnT_t[f][:], fnT_view[:, f * KF : (f + 1) * KF])

            # enqueue the whole 8 MB input stream up-front, with the issue
            # cost (~600ns per dma_start) spread over the three DGE-capable
            # queues; the first tiles are in flight while the PE is still
            # warming up
            mt = {}

            def dma_mt(h, k, eng):
                t = mt_pool.tile(
                    [128, PJ, pass_width], mmdt, tag="mt", name=f"mt_{h}_{k}"
                )
                # host layout groups [h][j][r] per row, so this DMA reads
                # one contiguous PJ*pass_width run per partition
                eng.dma_start(
                    t[:],
                    mT_d[k * 128 : (k + 1) * 128, :].rearrange(
                        "p (h j r) -> p h j r", h=NH, j=PJ
                    )[:, h],
                )
                mt[h, k] = t

            # single-queue issue in consumption order: SP's ~0.6us per
            # dma_start stays ahead of the PE's ~0.85us per tile, tiles
            # complete strictly in order, and no second DGE queue competes
            # for the rings on the critical first tiles. With one pool
            # buffer per tile no dma_start carries a wait. GpSimd SW-DGE is
            # avoided entirely (its drain costs ~7us in the epilogue).
            dma_fnT(0, nc.sync)
            dma_mt(0, 0, nc.sync)
            dma_fnT(1, nc.sync)
            dma_mt(0, 1, nc.sync)
            dma_mt(0, 2, nc.sync)
            dma_fnT(2, nc.sync)
            dma_mt(0, 3, nc.sync)
            dma_fnT(3, nc.sync)
            for h in range(NH):
                for k in range(KC):
                    if h == 0 and k < 4:
                        continue
                    dma_mt(h, k, nc.sync)

            pending_out = []
            for h in range(NH):
                ps = {}
                for g in range(NG):
                    for rs in range(RS):
                        ps[g, rs] = psum_pool.tile(
                            [128, 512], dt.float32, tag=f"ps{g}_{rs}",
                            name=f"ps{g}_{rs}_{h}",
                        )
                outs = [
                    out_pool.tile(
                        [128, PWP], outdt, tag=f"out{g}", name=f"out{g}_{h}"
                    )
                    for g in range(NG)
                ]
                for k in range(KC):
                    # rs-major: both rs=0 banks retire two matmuls before the
                    # pass ends, so the pool copies overlap the last matmuls
                    for rs in range(RS):
                        for g in range(NG):
                            if double_row:
                                lhsT = fnT_t[k // KF][
                                    :, k % KF, :, g * 128 : (g + 1) * 128
                                ]
                                rhs = mt[h, k][:, :, rs * 512 : (rs + 1) * 512]
                            else:
                                lhsT = fnT_t[k // KF][
                                    :, k % KF, 0, g * 128 : (g + 1) * 128
                                ]
                                rhs = mt[h, k][:, 0, rs * 512 : (rs + 1) * 512]
                            nc.tensor.matmul(
                                ps[g, rs][:],
                                lhsT,
                                rhs,
                                start=(k == 0),
                                stop=(k == KC - 1),
                                perf_mode=perf_mode,
                            )
                for g in range(NG):
                    # 4:1 max-pool as the evacuation. The ISA allows at most
                    # one PSUM operand per DVE op, so the host interleaves
                    # each quad across the two banks of the pass: quad p sits
                    # at bank0[p], bank1[p], bank0[p+256], bank1[p+256].
                    # Copies are halved on ACT so the DVE max chain starts
                    # as early as possible (GpSimd cannot read PSUM).
                    t = tmp_pool.tile([128, 512], dt.float32, tag="t",
                                      name=f"t{g}_{h}")
                    u = tmp_pool.tile([128, 512], dt.float32, tag="u",
                                      name=f"u{g}_{h}")
                    nc.scalar.copy(t[:, 0:PWP], ps[g, 0][:, 0:PWP])
                    nc.scalar.copy(t[:, PWP : 2 * PWP], ps[g, 0][:, PWP : 2 * PWP])
                    nc.vector.tensor_max(
                        u[:, 0:PWP], ps[g, 1][:, 0:PWP], t[:, 0:PWP]
                    )
                    nc.vector.tensor_max(
                        u[:, PWP : 2 * PWP],
                        ps[g, 1][:, PWP : 2 * PWP],
                        t[:, PWP : 2 * PWP],
                    )
                    nc.vector.tensor_max(
                        outs[g][:], u[:, 0:PWP], u[:, PWP : 2 * PWP]
                    )

                    # the last pass's two fires are the tail: split them over
                    # the two HW-DGE queues so they issue in parallel
                    feng = nc.scalar if (h == NH - 1 and g == 0) else nc.sync

                    def fire(g=g, h=h, outs=outs, feng=feng):
                        feng.dma_start(
                            s0_d[g * 128 : (g + 1) * 128, h * PWP : (h + 1) * PWP],
                            outs[g][:],
                        )
                    pending_out.append(fire)
            # all output DMAs at the end of the SP stream: each waits only on
            # its own pools, so they execute mid-kernel without ever blocking
            # an input tile behind them
            for fire in pending_out:
                fire()
    return nc


def _ensure_ntff_hook():
    """bass_utils' trace path imports antenv.axon_hooks, which this image's
    antenv lacks. Provide the module and register the ctypes NTFF hook the
    boot would have installed."""
    import sys
    import types

    try:
        import antenv.axon_hooks  # noqa: F401

        return
    except ImportError:
        pass
    import antenv

    mod = types.ModuleType("antenv.axon_hooks")
    state = {"h": None}
    mod.set_axon_ntff_profile_hook = lambda h: state.__setitem__("h", h)
    mod.get_axon_ntff_profile_hook = lambda: state["h"]
    sys.modules["antenv.axon_hooks"] = mod
    antenv.axon_hooks = mod
    try:
        from trn_agent_boot.trn_boot import _ntff_profile_via_ctypes

        h = _ntff_profile_via_ctypes("/opt/axon/libaxon_pjrt.so")
        if h is not None:
            mod.set_axon_ntff_profile_hook(h)
    except Exception:
        pass


def _get_program():
    if "nc" not in _CACHE:
        _CACHE["nc"] = build_sims_program()
    return _CACHE["nc"]


def _mm_np_dtype():
    import ml_dtypes

    return ml_dtypes.float8_e4m3


def _prep_mT(m, mmnp, n_pass=4):
    """[L, D] memory shard -> [D/2, 2*L] device layout: row (k*128+p) holds
    [h][j][r] so each (h, k) tile DMA is one contiguous run per partition;
    logical row d = k*256 + j*128 + p."""
    Lc, Dd = m.shape
    pw = Lc // n_pass
    return np.ascontiguousarray(
        m.T.reshape(Dd // 256, 2, 128, n_pass, pw)
        .transpose(0, 2, 3, 1, 4)
        .reshape(Dd // 2, 2 * Lc),
        dtype=mmnp,
    )


def _quad_perm():
    """Device row permutation: quad p of a pass must land at bank0[p],
    bank1[p], bank0[p+256], bank1[p+256] so the two pooling maxes only ever
    read one PSUM operand. perm[dev_row] = original row."""
    if "perm" in _CACHE:
        return _CACHE["perm"]
    j = np.arange(512)
    off = 4 * (j % 256) + 2 * (j // 256)
    passperm = np.concatenate([off, off + 1])         # dev col rs*512+j
    perm = (np.arange(L // 1024)[:, None] * 1024 + passperm[None, :]).reshape(-1)
    _CACHE["perm"] = perm
    return perm


def _device_sims_pooled(fn, mem0, invn_full):
    """fn [B, D] normalized; mem0 [C, L, D]; invn_full [C*L] reciprocal row
    norms. Returns pooled [B, C*L/POOL_W] f32: quad-max of GAIN^2-scaled
    normalized similarities, matmul + pooling on the 8 NeuronCores."""
    from concourse.bass_utils import run_bass_kernel_spmd

    nc = _get_program()
    Bb, Dd = fn.shape
    mmnp = _mm_np_dtype()
    # [D, B] -> [KC, 2, 128, B] -> [128, KC, 2, B] -> [128, KC*2*B]
    # (logical row d = c*256 + j*128 + p, pre-chunked for one linear DMA)
    fnT = np.ascontiguousarray(
        (fn.T * GAIN)
        .reshape(Dd // 256, 2, 128, Bb)
        .transpose(2, 0, 1, 3)
        .reshape(128, -1),
        dtype=mmnp,
    )
    perm = _quad_perm()
    in_maps = []
    for c in range(N_CORES):
        mn = mem0[c] * (invn_full[c * L : (c + 1) * L, None] * GAIN)
        in_maps.append({"fnT": fnT, "mT": _prep_mT(mn[perm], mmnp)})
    import os

    kwargs = {}
    if os.environ.get("KERNEL_TRACE"):
        _ensure_ntff_hook()
        kwargs = {"trace": True, "trace_cores": [0]}
    res = run_bass_kernel_spmd(nc, in_maps, core_ids=list(range(N_CORES)), **kwargs)
    _CACHE["exec_time_ns"] = res.exec_time_ns
    _CACHE["trace"] = res.instructions_and_trace
    return np.concatenate(
        [res.results[c]["s0"].astype(np.float32) for c in range(N_CORES)], axis=1
    )


def _logsumexp(x, axis):
    m = np.max(x, axis=axis, keepdims=True)
    return m + np.log(np.sum(np.exp(x - m), axis=axis, keepdims=True))


def _pool_maps():
    """Index maps between original L-columns and pooled quad indices.

    Device pooled column q of core c covers pass h = (q % 1024) // 256,
    quad p = q % 256 -> original columns c*4096 + h*1024 + 4p + {0..3}.
    """
    if "quad_l0" in _CACHE:
        return _CACHE["quad_l0"], _CACHE["col_to_quad"]
    nq = C * L // POOL_W
    q = np.arange(nq, dtype=np.int64)
    c, u = q // (L // POOL_W), q % (L // POOL_W)
    h, p = u // 256, u % 256
    quad_l0 = c * L + h * 1024 + POOL_W * p
    col_to_quad = np.empty(C * L, dtype=np.int64)
    cols = quad_l0[:, None] + np.arange(POOL_W)[None, :]
    col_to_quad[cols.reshape(-1)] = np.repeat(q, POOL_W)
    _CACHE["quad_l0"] = quad_l0
    _CACHE["col_to_quad"] = col_to_quad
    return quad_l0, col_to_quad


def kernel(
    features,
    targets,
    cams,
    all_pseudo_label,
    all_img_cams,
    init_intra_id_feat,
    epoch,
    batch_ind,
):
    f = np.asarray(features, dtype=np.float32)
    targets = np.asarray(targets)
    cams = np.asarray(cams)
    mem0 = np.asarray(init_intra_id_feat, dtype=np.float32)   # [C, L, D]
    percam = B // C

    fn = f / np.linalg.norm(f, axis=1, keepdims=True)
    mflat = mem0.reshape(C * L, D)
    invn_full = 1.0 / np.sqrt(np.einsum("rd,rd->r", mflat, mflat))

    # --- heavy part on device: quad-max-pooled normalized sims ---
    pooled = _device_sims_pooled(fn, mem0, invn_full)         # [B, C*L/4]

    # --- EMA update (only its effect on the CE logits is needed) ---
    old = mem0[cams, targets]                                 # [B, D]
    new = ALPHA * old + (1.0 - ALPHA) * f
    new_n = new / np.linalg.norm(new, axis=1, keepdims=True)
    # memn rows get normalized once more in the reference; idempotent but
    # replicate for exactness of the patched columns
    new_n = new_n / np.linalg.norm(new_n, axis=1, keepdims=True)
    P = fn @ new_n.T                                          # [B, B]

    # --- per-camera proxy CE; the diagonal blocks are recomputed exactly on
    # host (2 GFLOP in BLAS), independent of the device result ---
    logits = np.empty((C, percam, L), dtype=np.float32)
    for c in range(C):
        blk = (
            fn[c * percam : (c + 1) * percam] @ mflat[c * L : (c + 1) * L].T
        ) * invn_full[None, c * L : (c + 1) * L]
        for j in np.nonzero(cams == c)[0]:                    # scatter order: last wins
            blk[:, targets[j]] = P[c * percam : (c + 1) * percam, j]
        logits[c] = blk
    logits /= BETA
    lsm = logits - _logsumexp(logits, axis=-1)
    t = targets.reshape(C, percam)
    ce = -np.take_along_axis(lsm, t[..., None], axis=-1)[..., 0]
    loss = ce.mean(axis=1).sum()

    # --- cross-camera associative loss ---
    # The device ships only pooled quad-maxima; positives and the BG_KNN
    # hardest negatives are recomputed exactly on host from a quad-expanded
    # shortlist selected with a safety margin.
    if int(epoch) >= CROSSCAM_EPOCH:
        QP = 192
        quad_l0, col_to_quad = _pool_maps()
        pos = targets[:, None] + np.arange(C, dtype=np.int64)[None, :] * L
        m_pos = mflat[pos.reshape(-1)].reshape(B, C, D)
        pos_sims = (
            np.matmul(m_pos, fn[:, :, None])[..., 0] * invn_full[pos]
        )                                                     # [B, C] exact
        cand_q = np.argpartition(-pooled, QP - 1, axis=1)[:, :QP]     # [B, QP]
        quads = np.concatenate([cand_q, col_to_quad[pos]], axis=1)    # [B, QP+C]
        cand = (
            quad_l0[quads][:, :, None] + np.arange(POOL_W, dtype=np.int64)
        ).reshape(B, -1)                                      # [B, (QP+C)*4]
        cand = np.sort(cand, axis=1)
        m_c = mflat[cand.reshape(-1)].reshape(B, cand.shape[1], D)
        cvals = (
            np.matmul(m_c, fn[:, :, None])[..., 0] * invn_full[cand]
        )                                                     # exact
        # mask duplicate columns (a positive's quad may also be in the
        # top-QP) and the positive columns themselves
        dup = np.zeros_like(cand, dtype=bool)
        dup[:, 1:] = cand[:, 1:] == cand[:, :-1]
        is_pos = (cand[:, :, None] == pos[:, None, :]).any(axis=2)
        cvals[dup | is_pos] = -np.inf
        topv = -np.sort(-cvals, axis=1)[:, :BG_KNN]
        cat = np.concatenate([pos_sims / BETA, topv / BETA], axis=1).astype(
            np.float32
        )
        ls2 = cat - _logsumexp(cat, axis=1)
        per = -ls2[:, :C].sum(axis=1) / C
        loss = loss + 0.5 * per.reshape(C, percam).mean(axis=1).sum()

    return np.asarray([loss], dtype=np.float32)
